# revision 13
# baseline (speedup 1.0000x reference)
"""Trainium2 Bass kernel for nn_AttnGreedySearch (attn greedy top-1 search).

Math restructure (exact in exact arithmetic):
  With A_t = W_k^t and c_t = b_k @ sum_{i<t} W_k^i (row form), the iterated
  corpus is ic_t = ic0 @ A_t + c_t where ic0 = X @ W_proj + b_proj.  Define
  the per-sample query column u~_j = A_{j+1} @ S_j with S_j = user + sum v_i
  (unnormalized running sum; positive scale + constant shift never change the
  argmax; softmax is monotonic so it is argmax-irrelevant).  Per iteration j:
      score'_j[s] = <ic0[s, :], u~_j>          (argmax-equivalent scores)
      g_j = ic0[argmax]                         (one-hot select, 16-dim)
      v_j = A_{j+1}^T g_j + c_{j+1}             (output row, exact)
      u~_{j+1} = W_k u~_j + M_j g_j + d_j,      M_j = A_{j+2} A_{j+1}^T,
                                                d_j = A_{j+2} c_{j+1}

Performance structure (fp16, rel-err budget ~1.4e-2 < 2e-2, host-validated):
  - Host pre-casts X to fp16 and pre-transposes to XT [10, 101, B] with a
    ones row (bias folded into the contraction).  Halves HBM traffic and
    removes all on-device PE transposes of the corpus.
  - P1: per item-tile matmul(lhsT=XT chunk [101,128] stationary, rhs=waug
    [101,16] moving) -> psum [128,16] is ALREADY sample-major; one ACT copy
    per 128-sample tile lands ic0a fp16 in SBUF.  LDW-bound (~64cyc FWL).
  - P2 per group of 1024 samples (8 blocks of 128 on the free axis), five
    chained iterations; groups run in staggered pairs so two dependency
    chains interleave.  prod/select are fp16 2x-mode DVE tensor-tensor ops,
    the score reduce is a DVE pool / GPSIMD reduce (alternating), the one-hot
    expansion rides ACT, the 10->1 select reduce is a pairwise fp16 TT tree,
    and the 16-dim recurrence stays on PE as 128x128 block-diagonal fp16
    matmuls.
  - All DMA is batched into ~28 large 3D-AP transfers (dma_start costs
    ~625ns of SP sequencer time each).
"""

import numpy as np

import concourse.bass as bass
import concourse.mybir as mybir
import concourse.tile as tile
from concourse import bacc
from concourse.bass_utils import run_bass_kernel_spmd
from concourse.masks import make_identity

F32 = mybir.dt.float32
F16 = mybir.dt.float16
SEARCH_NUM = 5
NCORES = 8
D = 100   # item feature dim
DA = D + 1
NSI = 10  # items per sample
H = 16    # projected dim
SH = NSI * H  # 160


def _host_constants(W_proj, b_proj, W_k, b_k):
    Wk = W_k.astype(np.float64)
    bk = b_k.astype(np.float64)
    A = [np.eye(H)]
    for _ in range(SEARCH_NUM + 1):
        A.append(A[-1] @ Wk)
    c = [np.zeros(H)]
    for _ in range(SEARCH_NUM + 1):
        c.append(c[-1] @ Wk + bk)

    def blkdiag8(m):
        out = np.zeros((128, 128))
        for t in range(8):
            out[t * H:(t + 1) * H, t * H:(t + 1) * H] = m
        return out.astype(np.float16)

    # blks packed [128, 10*128] fp16: wk, a0..a4, m0..m3
    blks = [blkdiag8(Wk.T)]
    for j in range(SEARCH_NUM):
        blks.append(blkdiag8(A[j + 1]))
    for j in range(SEARCH_NUM - 1):
        Mj = A[j + 2] @ A[j + 1].T
        blks.append(blkdiag8(Mj.T))
    blks = np.concatenate(blks, axis=1)  # [128, 1280] fp16

    # cv0..cv4, dv0..dv3 packed [128, 9] fp32
    cvdv = []
    for j in range(SEARCH_NUM):
        cvdv.append(np.tile(c[j + 1], 8).astype(np.float32)[:, None])
    for j in range(SEARCH_NUM - 1):
        dj = A[j + 2] @ c[j + 1]
        cvdv.append(np.tile(dj, 8).astype(np.float32)[:, None])
    cvdv = np.concatenate(cvdv, axis=1)  # [128, 9] fp32

    waug = np.zeros((DA, H), dtype=np.float16)
    waug[:D, :] = W_proj.astype(np.float16)
    waug[D, :] = b_proj.astype(np.float16)
    return {"blks": blks, "cvdv": cvdv, "waug": waug}


def _v(t, off, dims, nparts=None):
    """View on tile/AP t: free dims `dims`, element offset `off` added.
    `nparts` overrides the partition count (step preserved)."""
    p = list(t.ap[0])
    if nparts is not None:
        p = [p[0], nparts]
    return bass.AP(tensor=t.tensor, offset=t.offset + off,
                   ap=[p] + [list(d) for d in dims])


def build_program(nc, B):
    assert B % 1024 == 0
    NT = B // 128
    NST = B // 512
    NG = B // 1024
    mult = mybir.AluOpType.mult
    add = mybir.AluOpType.add
    iseq = mybir.AluOpType.is_equal

    xt_d = nc.dram_tensor("xt", [NSI, DA, B], F16, kind="ExternalInput").ap()
    user_d = nc.dram_tensor("user", [B, H], F32, kind="ExternalInput").ap()
    waug_d = nc.dram_tensor("waug", [DA, H], F16, kind="ExternalInput").ap()
    blks_d = nc.dram_tensor("blks", [128, 1280], F16, kind="ExternalInput").ap()
    cvdv_d = nc.dram_tensor("cvdv", [128, 9], F32, kind="ExternalInput").ap()
    out_d = nc.dram_tensor("out", [B, SEARCH_NUM + 1, H], F32,
                           kind="ExternalOutput").ap()

    with tile.TileContext(nc) as tc:
        with tc.tile_pool(name="singles", bufs=1) as singles, \
             tc.tile_pool(name="xst", bufs=3) as xst, \
             tc.tile_pool(name="scr", bufs=3) as scr, \
             tc.tile_pool(name="vop", bufs=2) as vop, \
             tc.tile_pool(name="ppr", bufs=3, space="PSUM") as ppr, \
             tc.tile_pool(name="pp2", bufs=5, space="PSUM") as pp2:

            # ---- persistent SBUF ----
            ident16 = singles.tile([128, 128], F16)
            make_identity(nc, ident16)
            ident32 = singles.tile([128, 128], F32)
            make_identity(nc, ident32)
            blks_sb = singles.tile([128, 1280], F16)
            nc.sync.dma_start(out=blks_sb, in_=blks_d)
            cvdv_sb = singles.tile([128, 9], F32)
            nc.sync.dma_start(out=cvdv_sb, in_=cvdv_d)
            waug_sb = singles.tile([DA, H], F16)
            nc.sync.dma_start(out=waug_sb, in_=waug_d)

            def blk_wk():
                return blks_sb[:, 0:128]

            def blk_a(j):
                return blks_sb[:, (1 + j) * 128:(2 + j) * 128]

            def blk_m(j):
                return blks_sb[:, (6 + j) * 128:(7 + j) * 128]

            def cv(j):
                return cvdv_sb[:, j:j + 1]

            def dv(j):
                return cvdv_sb[:, 5 + j:6 + j]

            ic0a = singles.tile([128, NT * SH], F16)   # (t,i,h) per tile
            usera = singles.tile([128, NG * 128], F32)  # sample-major (t,h)
            ua = singles.tile([128, NG * 128], F16)     # u~ sample-major
            ud = singles.tile([128, NG * 128], F16)     # u~ feature-major

            # one big user DMA: usera[p, (g,t,h)] = user[g*1024+t*128+p, h]
            src_ap = bass.AP(tensor=user_d.tensor, offset=user_d.offset,
                             ap=[[H, 128], [128 * H, NG * 8], [1, H]])
            nc.sync.dma_start(out=usera, in_=src_ap)

            # ---- P0: u~_0 = W_k @ user per group ----
            for g in range(NG):
                tp = pp2.tile([128, 128], F32, name="tp0", tag="p2")
                nc.tensor.transpose(tp, usera[:, g * 128:(g + 1) * 128],
                                    ident32)
                userd16 = scr.tile([128, 128], F16, name="userd16", tag="gd")
                nc.scalar.copy(userd16, tp)
                up = pp2.tile([128, 128], F32, name="up0", tag="p2")
                nc.tensor.matmul(up, blk_wk(), userd16, start=True, stop=True)
                nc.scalar.copy(ud[:, g * 128:(g + 1) * 128], up)
                tp2 = pp2.tile([128, 128], F16, name="tp0b", tag="p2")
                nc.tensor.transpose(tp2, ud[:, g * 128:(g + 1) * 128],
                                    ident16)
                nc.scalar.copy(ua[:, g * 128:(g + 1) * 128], tp2)

            # ---- P1 for one super-tile of 512 samples ----
            def emit_p1(st):
                xt_sb = xst.tile([DA, NSI * 512], F16, name="xt_sb")
                src = bass.AP(tensor=xt_d.tensor, offset=xt_d.offset + st * 512,
                              ap=[[B, DA], [DA * B, NSI], [1, 512]])
                nc.sync.dma_start(out=xt_sb, in_=src)
                for a in range(4):
                    c_ = st * 4 + a
                    pc = ppr.tile([128, SH], F32, name="pc", tag="pc")
                    for i in range(NSI):
                        nc.tensor.matmul(
                            pc[:, i * H:(i + 1) * H],
                            xt_sb[:, i * 512 + a * 128:i * 512 + (a + 1) * 128],
                            waug_sb, start=True, stop=True)
                    nc.scalar.copy(ic0a[:, c_ * SH:(c_ + 1) * SH], pc)

            # ---- P2: one iteration for one group ----
            def emit_iter(g, j, score_on_gps):
                base = g * 8 * SH
                ua_sl = _v(ua, g * 128, [[H, 8], [0, NSI], [1, H]])
                ud_sl = ud[:, g * 128:(g + 1) * 128]
                ic_g = _v(ic0a, base, [[SH, 8], [H, NSI], [1, H]])

                prod = scr.tile([128, 8, NSI, H], F16, name="prod", tag="prod")
                nc.vector.tensor_tensor(out=prod, in0=ic_g, in1=ua_sl, op=mult)
                scores = scr.tile([128, 8, NSI], F32, name="scores",
                                  tag="scores")
                nc.vector.reduce_sum(out=scores, in_=prod,
                                     axis=mybir.AxisListType.X)
                mx = scr.tile([128, 8], F32, name="mx", tag="mx")
                nc.vector.reduce_max(out=mx, in_=scores,
                                     axis=mybir.AxisListType.X)
                mask = scr.tile([128, 8, NSI], F32, name="mask", tag="mask")
                nc.vector.tensor_tensor(
                    out=mask, in0=scores, in1=_v(mx, 0, [[1, 8], [0, NSI]]),
                    op=iseq)
                mask16 = scr.tile([128, 8, NSI, H], F16, name="mask16",
                                  tag="mask16")
                nc.scalar.copy(mask16, _v(mask, 0,
                                          [[NSI, 8], [1, NSI], [0, H]]))
                sel = scr.tile([128, 8, NSI, H], F16, name="sel", tag="sel")
                sel_eng = nc.gpsimd if score_on_gps else nc.vector
                sel_eng.tensor_tensor(out=sel, in0=ic_g, in1=mask16, op=mult)
                # pairwise fold over items: 10 -> 5 -> (4->2->1) + leftover
                f1 = scr.tile([128, 8, 5, H], F16, name="f1", tag="f1")
                nc.gpsimd.tensor_tensor(
                    out=f1, in0=_v(sel, 0, [[SH, 8], [H, 5], [1, H]]),
                    in1=_v(sel, 5 * H, [[SH, 8], [H, 5], [1, H]]), op=add)
                f2 = scr.tile([128, 8, 2, H], F16, name="f2", tag="f2")
                nc.vector.tensor_tensor(
                    out=f2, in0=_v(f1, 0, [[5 * H, 8], [H, 2], [1, H]]),
                    in1=_v(f1, 2 * H, [[5 * H, 8], [H, 2], [1, H]]), op=add)
                f3 = scr.tile([128, 8, H], F16, name="f3", tag="f3")
                nc.vector.tensor_tensor(
                    out=f3, in0=_v(f2, 0, [[2 * H, 8], [1, H]]),
                    in1=_v(f2, H, [[2 * H, 8], [1, H]]), op=add)
                ga = scr.tile([128, 8, H], F16, name="ga", tag="ga")
                nc.vector.tensor_tensor(
                    out=ga, in0=f3, in1=_v(f1, 4 * H, [[5 * H, 8], [1, H]]),
                    op=add)

                tpg = pp2.tile([128, 128], F16, name="tpg", tag="p2")
                nc.tensor.transpose(tpg, ga, ident16)
                gd16 = scr.tile([128, 128], F16, name="gd16", tag="gd")
                nc.scalar.copy(gd16, tpg)
                # u~ recurrence first: cross-iteration critical path
                if j < SEARCH_NUM - 1:
                    up = pp2.tile([128, 128], F32, name="upj", tag="p2")
                    nc.tensor.matmul(up, blk_wk(), ud_sl, start=True,
                                     stop=False)
                    nc.tensor.matmul(up, blk_m(j), gd16, start=False,
                                     stop=True)
                    nc.vector.tensor_scalar_add(ud_sl, up, dv(j))
                    tpu = pp2.tile([128, 128], F16, name="tpu", tag="p2")
                    nc.tensor.transpose(tpu, ud_sl, ident16)
                    nc.scalar.copy(ua[:, g * 128:(g + 1) * 128], tpu)
                vp = pp2.tile([128, 128], F32, name="vp", tag="p2")
                nc.tensor.matmul(vp, blk_a(j), gd16, start=True, stop=True)
                vtmp = scr.tile([128, 128], F16, name="vtmp", tag="vtmp")
                nc.vector.tensor_scalar_add(vtmp, vp, cv(j))
                tpv = pp2.tile([128, 128], F16, name="tpv", tag="p2")
                nc.tensor.transpose(tpv, vtmp, ident16)
                vout = vouts[g % 2]
                nc.scalar.copy(_v(vout, (1 + j) * H, [[96, 8], [1, H]]), tpv)
                if j == 0:
                    nc.scalar.copy(_v(vout, 0, [[96, 8], [1, H]]),
                                   _v(usera, g * 128, [[H, 8], [1, H]]))

            def emit_p3(g):
                vout = vouts[g % 2]
                dst = bass.AP(
                    tensor=out_d.tensor,
                    offset=out_d.offset + g * 1024 * 96,
                    ap=[[96, 128], [128 * 96, 8], [1, 96]])
                nc.sync.dma_start(out=dst, in_=vout)

            vouts = [vop.tile([128, 8 * 96], F32, name=f"vout{k}")
                     for k in range(2)]

            # ---- main schedule: P1 pairs ahead, P2 staggered group pairs ----
            emit_p1(0)
            emit_p1(1)
            emit_p1(2)
            emit_p1(3)
            for q in range(NG // 2):
                gA, gB = 2 * q, 2 * q + 1
                for j in range(SEARCH_NUM):
                    emit_iter(gA, j, score_on_gps=True)
                    emit_iter(gB, j, score_on_gps=False)
                    # feed P1 of the next pair between iterations
                    nst = 4 * (q + 1) + j
                    if j < 4 and nst < NST:
                        emit_p1(nst)
                emit_p3(gA)
                emit_p3(gB)


def _in_maps(inputs, B_core):
    cst = _host_constants(inputs["W_proj"], inputs["b_proj"],
                          inputs["W_k"], inputs["b_k"])
    x = np.asarray(inputs["item_corpus"], dtype=np.float32)
    u = np.ascontiguousarray(inputs["user_intent"], dtype=np.float32)
    B = x.shape[0]
    # XT [10, 101, B] fp16 with ones row (bias contraction row)
    xt = np.empty((NSI, DA, B), dtype=np.float16)
    xt[:, :D, :] = x.transpose(1, 2, 0).astype(np.float16)
    xt[:, D, :] = 1.0
    maps = []
    for core in range(NCORES):
        lo, hi = core * B_core, (core + 1) * B_core
        m = {"xt": np.ascontiguousarray(xt[:, :, lo:hi]),
             "user": u[lo:hi],
             "waug": cst["waug"], "blks": cst["blks"], "cvdv": cst["cvdv"]}
        maps.append(m)
    return maps


_COMPILED = {}


def _get_nc(B_core):
    if B_core not in _COMPILED:
        nc = bacc.Bacc("TRN2", target_bir_lowering=False, debug=False,
                       num_devices=NCORES)
        build_program(nc, B_core)
        nc.compile()
        _COMPILED[B_core] = nc
    return _COMPILED[B_core]


def kernel(**inputs) -> np.ndarray:
    bs = inputs["user_intent"].shape[0]
    assert bs % NCORES == 0
    B_core = bs // NCORES
    nc = _get_nc(B_core)
    res = run_bass_kernel_spmd(nc, _in_maps(inputs, B_core),
                               core_ids=list(range(NCORES)))
    out = np.concatenate([r["out"] for r in res.results], axis=0)
    return out.astype(np.float32)
